# revision 68
# baseline (speedup 1.0000x reference)
"""Trainium2 Bass kernel for MinEuclideanDistBlockACS (retrieval_knn).

Computes, for x:(64,3,4096) f32 and shapelets:(3,128,64) f32:
  d[n,0,k] = min over channels c and windows w of
             || x[n,c,w:w+64] - shapelets[c,k,:] ||_2
i.e. the reference's unfold -> cdist -> min over windows -> min over channels.

Strategy (data-parallel over batch N across 8 NeuronCores, 8 samples/core,
no cross-core communication; outputs are concatenated on host):

  * For each (sample, channel) a Hankel (im2col) tile H is built in SBUF:
    rows s=0..63 hold x shifted by s (overlapping HWDGE DMA reads straight
    from DRAM), row 64 holds the sliding squared-norm
    x2[w] = sum_s x[w+s]^2.
  * One PE matmul per window chunk with lhsT = [-2*shapelets_c ; 1] then
    yields t[k,w] = x2[w] - 2*<x_w, s_k> for all 128 shapelets directly in
    PSUM.  d^2 = t + ||s_k||^2 is formed after the window-min (the ||s_k||^2
    term is window-independent).
  * Inputs use float32r (single-pass TF32-like PE path, 4x faster than the
    two-pass fp32 path).  x is pre-rounded to fp32r once in DRAM by a
    DRAM->DRAM cast DMA so the Hankel loads need no per-tile casting.
  * x2 is computed on-chip with two tiny PE matmuls against triangular
    prefix/suffix matrices (PE transpose -> ACT square -> prefix matmuls),
    then flattened into the Hankel row with one SBUF->SBUF DMA.
  * PSUM evacuation is split across engines (PSUM has a single DVE read
    port): DVE tensor_reduce(min) handles some chunks directly; ScalarE
    copies the others to SBUF where a single-source DVE
    tensor_scalar(min, accum_out=min) runs in 2x mode.
  * Final per-sample combine: + ||s_k||^2, min over channels, relu, sqrt,
    then a PE transpose emits the (8,128) per-core output in one DMA.
"""

import sys

import numpy as np

for _p in ("/opt/trn_rl_repo",):
    if _p not in sys.path:
        sys.path.insert(0, _p)

import concourse.bass as bass
import concourse.tile as tile
from concourse import bacc, mybir
from concourse.bass_utils import run_bass_kernel_spmd

F32 = mybir.dt.float32

N, C, L = 64, 3, 4096
K, S = 128, 64
W = L - S + 1  # 4033 windows
NCORES = 8
NPC = N // NCORES  # samples per core
PAD = 128  # tail padding for overlapping hankel reads
CHUNK = 1024  # window chunk per PSUM tile (2 banks)
NCHUNK = (W + CHUNK - 1) // CHUNK  # 4
XR_ROW = (NPC * C * L + PAD) // 128  # 769: rows for the fp32r pre-round pass
BIG = 1.0e30  # mask value making invalid windows lose every min

_CACHE = {}


def _build_bass():
    # Bacc (not raw Bass): its finalize() runs the TRN2 sync legalization
    # passes (move_matmul_waits_to_ldweights / generate_event_semaphores) --
    # walrus rejects instructions carrying more than one semaphore wait.
    nc = bacc.Bacc("TRN2", target_bir_lowering=False, debug=False)

    x_d = nc.dram_tensor("xbuf", (NPC * C * L + PAD,), F32, kind="ExternalInput")
    xr_d = nc.dram_tensor("xr", (NPC * C * L + PAD,), mybir.dt.float32r, kind="Internal")
    lhsT_d = nc.dram_tensor("lhsT", (S + 1, C * K), F32, kind="ExternalInput")
    s2_d = nc.dram_tensor("s2t", (K, C), F32, kind="ExternalInput")
    m1m2_d = nc.dram_tensor("m1m2", (S, 2 * S), F32, kind="ExternalInput")
    ident_d = nc.dram_tensor("ident", (K, K), F32, kind="ExternalInput")
    out_d = nc.dram_tensor("out", (NPC, K), F32, kind="ExternalOutput")

    with tile.TileContext(nc) as tc:
        with (
            tc.tile_pool(name="consts", bufs=1) as consts,
            tc.tile_pool(name="hankp", bufs=4) as hankp,
            tc.tile_pool(name="smallp", bufs=3) as smallp,
            tc.tile_pool(name="redp", bufs=2) as redp,
            tc.tile_pool(name="outp", bufs=1) as outp,
            tc.tile_pool(name="ps_small", bufs=1, space="PSUM") as ps_small,
            tc.tile_pool(name="ps_main", bufs=3, space="PSUM") as ps_main,
            tc.tile_pool(name="ps_out", bufs=1, space="PSUM") as ps_out,
        ):
            # float32r tiles: SWDGE cast-DMA performs the required fp32->fp32r
            # rounding so the fast single-pass PE matmul path is legal.
            F32R = mybir.dt.float32r
            lhsT_sb = consts.tile([S + 1, C * K], F32R)
            nc.gpsimd.dma_start(lhsT_sb[:, :], lhsT_d[:, :])

            # Pre-round all of x to fp32r once in DRAM (DRAM->DRAM cast DMA),
            # so every Hankel row load is a cast-free HWDGE DMA.
            # split into 4 pieces so the first Hankel loads only wait on
            # the first piece instead of the whole 393KB cast
            NPIECE = 4
            PROWS = 128 // NPIECE
            for pc in range(NPIECE):
                off = pc * PROWS * XR_ROW
                nc.gpsimd.dma_start(
                    bass.AP(tensor=xr_d[:].tensor, offset=off, ap=[[XR_ROW, PROWS], [1, XR_ROW]]),
                    bass.AP(tensor=x_d[:].tensor, offset=off, ap=[[XR_ROW, PROWS], [1, XR_ROW]]),
                )
            s2_sb = consts.tile([K, C], F32)
            nc.sync.dma_start(s2_sb[:, :], s2_d[:, :])
            m1m2_sb = consts.tile([S, 2 * S], F32)
            nc.sync.dma_start(m1m2_sb[:, :], m1m2_d[:, :])
            ident_sb = consts.tile([K, K], F32)
            nc.sync.dma_start(ident_sb[:, :], ident_d[:, :])

            x_ap = x_d[:]
            outT = outp.tile([K, NPC], F32)  # column n = result for sample n
            dminAll = outp.tile([K, NPC], F32)  # clamped d^2 mins per sample
            # prefetch the ACT sqrt table set during lead-in slack so the
            # single batched sqrt at the end pays no table-load stall
            sqd = smallp.tile([K, 1], F32, tag="sqd")
            nc.scalar.sqrt(sqd[:, :], ident_sb[:, 0:1])

            # Software pipeline: the x2/Hankel build chain for iteration
            # idx+2 is emitted before the matmul+reduce stage of iteration
            # idx, so the PE->ACT->PE->ACT x2 chain fully hides under the
            # previous iterations' PSUM evacuation.
            live_hank = {}
            chan_tiles = {}

            def build_chain(idx):
                base = idx * L
                # x2[w] (sliding squared norms) via prefix matmuls; small
                # DMAs ride the ACT HWDGE queue so the big Hankel loads on
                # the SP queue are never blocked behind them.
                compact = smallp.tile([S, S], F32, tag="compact")
                nc.scalar.dma_start(
                    compact[:, :],
                    bass.AP(tensor=x_ap.tensor, offset=base, ap=[[S, S], [1, S]]),
                )
                ps_t = ps_small.tile([S, S], F32, tag="pssm")
                nc.tensor.transpose(ps_t[:, :], compact[:, :], ident_sb[0:S, 0:S])
                xsqT = smallp.tile([S, S], F32, tag="xsqT")
                nc.scalar.activation(
                    xsqT[:, :], ps_t[:, :], mybir.ActivationFunctionType.Square
                )
                x2ps = ps_small.tile([S, S], F32, tag="pssm")
                nc.tensor.matmul(
                    x2ps[:, :], xsqT[:, :], m1m2_sb[:, 0:S],
                    start=True, stop=False,
                )
                nc.tensor.matmul(
                    x2ps[0 : S - 1, :], xsqT[:, 1:S], m1m2_sb[:, S : 2 * S],
                    start=False, stop=True, skip_group_check=True,
                )
                x2sb = smallp.tile([S, S], F32R, tag="x2sb")
                nc.scalar.copy(x2sb[:, :], x2ps[:, :])

                # Hankel tile: rows 0..63 shifted x (pre-rounded fp32r),
                # row 64 = x2 flattened (64,64) -> (1,4096).  Columns beyond
                # W carry garbage x2; the only chunk covering them bounds
                # its reduce to the valid columns.
                hank = hankp.tile([S + 1, L], F32R)
                # alternate the two HWDGE rings (SP / ACT) so consecutive
                # 1MB Hankel loads overlap their completion round-trips
                hank_eng = nc.sync if idx % 2 == 0 else nc.scalar
                hank_eng.dma_start(
                    hank[0:S, :],
                    bass.AP(tensor=xr_d[:].tensor, offset=base, ap=[[1, S], [1, L]]),
                )
                nc.scalar.dma_start(hank[S : S + 1, 0:L], x2sb[:, :])
                live_hank[idx] = hank

            def main_stage(idx):
                n, c = divmod(idx, C)
                hank = live_hank.pop(idx)
                if c == 0:
                    chan3 = smallp.tile([K, C], F32, tag="chan3")
                    chan_tiles[n] = chan3
                chan3 = chan_tiles[n]

                lhsT_r = lhsT_sb[:, c * K : (c + 1) * K]
                partials = redp.tile([K, NCHUNK], F32, tag="partials")
                # PSUM evacuation split between DVE and ScalarE: DVE
                # tensor_reduce reads PSUM at 1 elem/cycle (single read
                # port); ScalarE copies the other chunks to SBUF where a
                # single-source DVE tensor_scalar(min, accum=min) runs in 2x
                # mode. ~2.7 of 4 chunks via ACT balances the engines.
                n_act = 2
                for ch in list(range(n_act, NCHUNK)) + list(range(n_act)):
                    w0 = ch * CHUNK
                    mps = ps_main.tile([K, CHUNK], F32, tag="mps")
                    nc.tensor.matmul(
                        mps[:, 0:512],
                        lhsT_r,
                        hank[:, w0 : w0 + 512],
                        start=True, stop=True,
                    )
                    nc.tensor.matmul(
                        mps[:, 512:1024],
                        lhsT_r,
                        hank[:, w0 + 512 : w0 + 1024],
                        start=True, stop=True,
                    )
                    if ch >= n_act:  # direct DVE reduce from PSUM
                        # last chunk: only w < W=4033 are real windows
                        valid = min(CHUNK, W - w0)
                        nc.vector.tensor_reduce(
                            partials[:, ch : ch + 1],
                            mps[:, 0:valid],
                            axis=mybir.AxisListType.X,
                            op=mybir.AluOpType.min,
                        )
                    else:  # ACT evacuates; DVE min-accums at 2x from SBUF
                        cp = redp.tile([K, CHUNK], F32, tag=f"cp{ch}")
                        nc.scalar.copy(cp[:, :], mps[:, :])
                        junk = redp.tile([K, CHUNK], F32, tag="tsjunk")
                        nc.vector.tensor_scalar(
                            junk[:, :],
                            cp[:, :],
                            BIG,
                            None,
                            op0=mybir.AluOpType.min,
                            op1=mybir.AluOpType.min,
                            accum_out=partials[:, ch : ch + 1],
                        )
                nc.vector.tensor_reduce(
                    chan3[:, c : c + 1],
                    partials[:, :],
                    axis=mybir.AxisListType.X,
                    op=mybir.AluOpType.min,
                )
                if c == C - 1:
                    # d = sqrt(relu(min_c (chan3 + s2)))
                    tmp3 = smallp.tile([K, C], F32, tag="tmp3")
                    nc.vector.tensor_add(tmp3[:, :], chan3[:, :], s2_sb[:, :])
                    # relu commutes with min: min_i max(x_i,0) = max(min_i x_i, 0)
                    junk3 = smallp.tile([K, C], F32, tag="junk3")
                    nc.vector.tensor_scalar(
                        junk3[:, :], tmp3[:, :], 0.0, None,
                        op0=mybir.AluOpType.max,
                        op1=mybir.AluOpType.min,
                        accum_out=dminAll[:, n : n + 1],
                    )
                    del chan_tiles[n]

            PIPE = 2
            TOT = NPC * C
            for step in range(TOT + PIPE):
                if step >= PIPE:
                    main_stage(step - PIPE)
                if step < TOT:
                    build_chain(step)

            # one batched sqrt for all samples (table already resident)
            nc.scalar.sqrt(outT[:, :], dminAll[:, :])

            # ---- transpose (K, NPC) -> (NPC, K) and store
            ps_o = ps_out.tile([NPC, K], F32)
            nc.tensor.transpose(ps_o[:, :], outT[:, :], ident_sb[:, :])
            out_sb = outp.tile([NPC, K], F32)
            nc.scalar.copy(out_sb[:, :], ps_o[:, :])
            nc.sync.dma_start(out_d[:, :], out_sb[:, :])

    nc.finalize()  # Bacc.compile(): reg alloc + TRN2 sync-wait legalization
    return nc


def _host_consts(shapelets: np.ndarray):
    shp = np.asarray(shapelets, np.float32)
    lhsT = np.zeros((S + 1, C * K), np.float32)
    for c in range(C):
        lhsT[:S, c * K : (c + 1) * K] = -2.0 * shp[c].T  # rows s: -2*shapelets[c,k,s]
        lhsT[S, c * K : (c + 1) * K] = 1.0  # x2-row weight
    s2t = np.ascontiguousarray((shp * shp).sum(-1).T.astype(np.float32))  # (K, C)
    r = np.arange(S)
    m1 = (r[:, None] >= r[None, :]).astype(np.float32)  # suffix-sum matrix
    m2 = (r[:, None] < r[None, :]).astype(np.float32)  # prefix-sum matrix
    m1m2 = np.concatenate([m1, m2], axis=1)  # (S, 2S)
    ident = np.eye(K, dtype=np.float32)
    return lhsT, s2t, m1m2, ident


def kernel(x: np.ndarray, shapelets: np.ndarray, _trace: bool = False):
    x = np.asarray(x, np.float32)
    lhsT, s2t, m1m2, ident = _host_consts(shapelets)

    if "nc" not in _CACHE:
        _CACHE["nc"] = _build_bass()
    nc = _CACHE["nc"]

    in_maps = []
    for core in range(NCORES):
        shard = x[core * NPC : (core + 1) * NPC].ravel()
        xbuf = np.concatenate([shard, np.zeros(PAD, np.float32)])
        in_maps.append(
            {"xbuf": xbuf, "lhsT": lhsT, "s2t": s2t, "m1m2": m1m2, "ident": ident}
        )

    res = run_bass_kernel_spmd(
        nc, in_maps, core_ids=list(range(NCORES)), trace=_trace
    )
    _CACHE["last_result"] = res

    out = np.concatenate([res.results[i]["out"] for i in range(NCORES)], axis=0)
    return out.reshape(N, 1, K).astype(np.float32)
